# revision 27
# baseline (speedup 1.0000x reference)
"""FenrirNet (spiking CNN) Trainium2 kernel.

Data-parallel over batch: 16 images -> 8 NeuronCores x 2 images.
Per core, the T=20 scan is executed layer-phased:
  P1: conv1+spike+pool for all t (membrane m1 PSUM-resident, PE accumulates)
  P2: conv2+spike+pool for all t (m2 PSUM-resident, both images)
  P3: conv3+spike+pool for all t (m3 PSUM-resident)
  P4: quantized FC (integer-valued f32r weights, exact) + LIF readout
Convs use float32r (tf32-rate) matmuls with hi/lo mantissa-split operands, so
products are accurate to ~2^-22 — below the fp32 reordering noise floor.
Spike semantics are exact: r = relu(m - thr) (ACT), reset gate (r==0)*m (DVE),
pooled spike = (max4(r) > 0) (DVE+GPSIMD maxes on bf16 r, sign-exact).
"""
import numpy as np
from contextlib import ExitStack

# ---- model config (hardcoded; must match reference.py) ----
B, T = 16, 20
H = W = 128
C1, C2, C3 = 32, 64, 128
NCLS = 10
FC1_BITS = 4
FC1_MULT = 128.0
NCORES = 8
BL = B // NCORES  # images per core = 2

PITCH1 = 66 * 66 + 68          # padded s1 image pitch (+ tail for grp shift 66)
PITCH2 = 34 * 34 + 40          # padded s2 image pitch (+ tail for grp shift 34)
XPITCH = 130 * 130 + 12        # padded x image pitch (+ tail for dma overread)


def _mask13(a):
    return (a.view(np.uint32) & np.uint32(0xFFFFE000)).view(np.float32)


def _build_host_inputs(x, w1, w2, w3, fc_w, thr1, thr2, thr3, fc1_beta, lif_thr):
    """Precompute per-core HBM inputs (pure layout/quantization work)."""
    x = np.asarray(x, np.float32)
    w1 = np.asarray(w1, np.float32)
    w2 = np.asarray(w2, np.float32)
    w3 = np.asarray(w3, np.float32)
    fc_w = np.asarray(fc_w, np.float32)

    # exact replication of reference quantize_weight (round = half-even)
    n = float(2 ** (FC1_BITS - 1) - 1)
    scale = np.float32(np.abs(fc_w).max() / n)
    q = np.clip(np.round(fc_w / scale), -n, n).astype(np.float32)  # integer-valued

    splits = {}
    for name, wt in (("w2", w2), ("w3", w3)):
        wh = _mask13(wt)
        splits[name] = (wh, wt - wh)

    # conv1 plain-fp32 block-diag lhsT [36, 128]
    w1b = np.zeros((36, 128), np.float32)
    for g in range(4):
        for dy in range(3):
            for dx in range(3):
                w1b[g * 9 + dy * 3 + dx,
                    g * 32:(g + 1) * 32] = w1[:, 0, dy, dx]

    # conv2 lhsT, 3 dx-groups: SBUF [96, 2*3*64]: col = (s*3+dy)*64 + o
    # partition (grp=dx)*32 + c; mm(dy) covers taps (dy, dx=grp)
    w2b = np.zeros((96, 2 * 3 * 64), np.float32)
    for s, ws in enumerate(splits["w2"]):
        for dy in range(3):
            col = (s * 3 + dy) * 64
            for grp in range(3):
                w2b[grp * 32:(grp + 1) * 32, col:col + 64] = ws[:, :, dy, grp].T

    # conv3 lhsT, SBUF layout [128, 2*6*128]: col = (s*6+j)*128 + o
    w3b = np.zeros((128, 2 * 6 * 128), np.float32)
    for s, ws in enumerate(splits["w3"]):
        for j in range(6):
            dx = j % 3
            col = (s * 6 + j) * 128
            if j < 3:
                w3b[0:64, col:col + 128] = ws[:, :, 0, dx].T
                w3b[64:128, col:col + 128] = ws[:, :, 1, dx].T
            else:
                w3b[0:64, col:col + 128] = ws[:, :, 2, dx].T

    # FC: SBUF layout [128 ch, 256*10]: col = px*10 + cls; fc_in = ch*256 + px
    import ml_dtypes
    qwb = np.ascontiguousarray(
        q.reshape(NCLS, C3, 256).transpose(1, 2, 0).reshape(C3, 2560)
    ).astype(ml_dtypes.bfloat16)

    # thresholds (negated, per-partition layouts)
    nthr1 = np.tile(-np.asarray(thr1, np.float32), 4).reshape(128, 1)
    nthr2 = np.tile(-np.asarray(thr2, np.float32), 2).reshape(128, 1)
    nthr3 = (-np.asarray(thr3, np.float32)).reshape(128, 1)
    thrs = np.concatenate([nthr1, nthr2, nthr3], axis=1)  # [128, 3]

    beta = float(np.clip(np.asarray(fc1_beta, np.float32), 0.0, 1.0))
    lim = float(scale * FC1_MULT)
    consts = np.tile(np.array([[float(scale), lim, beta,
                                float(np.asarray(lif_thr))]], np.float32),
                     (128, 1))  # replicated per partition

    # padded x per core: xs[core] = [BL, T, XPITCH]
    xs = np.zeros((NCORES, BL, T, XPITCH), np.float32)
    for c in range(NCORES):
        for b in range(BL):
            for t in range(T):
                img = np.zeros((130, 130), np.float32)
                img[1:129, 1:129] = x[c * BL + b, t, 0]
                xs[c, b, t, :130 * 130] = img.reshape(-1)
    xs = xs.reshape(NCORES, BL * T * XPITCH)

    shared = {
        "w1b": w1b, "w2b": w2b, "w3b": w3b, "qwb": qwb,
        "thrs": thrs, "consts": consts,
    }
    return xs, shared


def _build_bass():
    import concourse.bass as bass
    import concourse.tile as tile
    import concourse.mybir as mybir
    import bass_rust
    _ns = {}
    exec(_WAITSPLIT_SRC, _ns)
    split_sync_waits = _ns["split_sync_waits"]

    dt = mybir.dt
    AF = mybir.ActivationFunctionType
    OP = mybir.AluOpType

    nc = bass.Bass(num_swdge_queues=4)
    xs_e = nc.declare_dram_parameter("xs", [BL * T * XPITCH], dt.float32,
                                     isOutput=False)
    w1b_e = nc.declare_dram_parameter("w1b", [36, 128], dt.float32, isOutput=False)
    w2b_e = nc.declare_dram_parameter("w2b", [96, 2 * 3 * 64], dt.float32,
                                      isOutput=False)
    w3b_e = nc.declare_dram_parameter("w3b", [128, 2 * 6 * 128], dt.float32,
                                      isOutput=False)
    qwb_e = nc.declare_dram_parameter("qwb", [128, 2560], dt.bfloat16,
                                      isOutput=False)
    thrs_e = nc.declare_dram_parameter("thrs", [128, 3], dt.float32, isOutput=False)
    con_e = nc.declare_dram_parameter("consts", [128, 4], dt.float32, isOutput=False)
    out_e = nc.declare_dram_parameter("out", [NCLS, T * BL], dt.float32, isOutput=True)

    s1d = nc.dram_tensor("s1d", [BL * T * C1 * PITCH1], dt.bfloat16)
    s2d = nc.dram_tensor("s2d", [BL * T * C2 * PITCH2 + 64], dt.bfloat16)

    def ap(tensor_ap, offset, dims):
        return bass_rust.AP(tensor=tensor_ap.tensor, offset=offset, ap=list(dims))

    with tile.TileContext(nc) as tc, ExitStack() as ctx:
        wpool = ctx.enter_context(tc.tile_pool(name="w", bufs=1))
        b1p = ctx.enter_context(tc.tile_pool(name="b1", bufs=2))
        b2p = ctx.enter_context(tc.tile_pool(name="b2", bufs=3))
        b3p = ctx.enter_context(tc.tile_pool(name="b3", bufs=3))
        rp = ctx.enter_context(tc.tile_pool(name="r", bufs=2))
        qp = ctx.enter_context(tc.tile_pool(name="q", bufs=2))
        sp = ctx.enter_context(tc.tile_pool(name="s", bufs=2))
        s3p = ctx.enter_context(tc.tile_pool(name="s3", bufs=1))
        psum = ctx.enter_context(tc.tile_pool(name="ps", bufs=1, space="PSUM"))

        # ---- static tiles (HBM already in SBUF layout; raw-bit DMAs) ----
        w1sb = wpool.tile([36, 128], dt.float32, tag="w1sb")
        nc.sync.dma_start(w1sb[:], w1b_e[:])
        w2sb = wpool.tile([96, 2 * 3 * 64], dt.float32r, tag="w2sb")
        nc.gpsimd.dma_start(w2sb[:], w2b_e[:])
        w3sb = wpool.tile([128, 2 * 6 * 128], dt.float32r, tag="w3sb")
        nc.gpsimd.dma_start(w3sb[:], w3b_e[:])
        qwsb = wpool.tile([128, 2560], dt.bfloat16, tag="qwsb")
        nc.sync.dma_start(qwsb[:], qwb_e[:])
        thrs = wpool.tile([128, 3], dt.float32, tag="thrs")
        nc.sync.dma_start(thrs[:], thrs_e[:])
        cons = wpool.tile([128, 4], dt.float32, tag="cons")
        nc.sync.dma_start(cons[:], con_e[:])
        s3_all = s3p.tile([128, 256 * T * BL], dt.bfloat16, tag="s3all")

        # zero the padded HBM scratch once (borders must be 0)
        zt = wpool.tile([128, 512], dt.bfloat16, tag="zt")
        nc.vector.memset(zt[:], 0.0)
        for buf, tot in ((s1d, BL * T * C1 * PITCH1),
                         (s2d, BL * T * C2 * PITCH2)):
            CH = 128 * 512
            for lo in range(0, tot, CH):
                n = min(CH, tot - lo)
                rows, rem = divmod(n, 512)
                if rows:
                    nc.sync.dma_start(ap(buf[:], lo, [[512, rows], [1, 512]]),
                                      zt[0:rows, :])
                if rem:
                    nc.sync.dma_start(ap(buf[:], lo + rows * 512, [[1, rem]]),
                                      zt[0:1, 0:rem])

        NQ = 8  # relu/reset split factor for pipelining

        # ================= P1: conv1 + spike/pool =================
        m1 = psum.tile([128, 4096], dt.float32, tag="mem")
        for img in range(BL):
            for t in range(T):
                xbase = (img * T + t) * XPITCH
                b1h = b1p.tile([36, 4164], dt.float32, tag="b1h")
                bt = b1h[:].rearrange("(g tp) c -> tp g c", tp=9)
                engs = (nc.gpsimd, nc.sync, nc.scalar)
                for dy in range(3):
                    for dx in range(3):
                        src = ap(xs_e[:], xbase + dy * 130 + dx,
                                 [[32 * 130, 4], [1, 4164]])
                        engs[(dy * 3 + dx) % 3].dma_start(bt[dy * 3 + dx], src)
                for n in range(8):
                    v = b1h[:, 520 * n:520 * n + 520]
                    rhs = v.rearrange("p (r x) -> p r x", x=130)[:, :, 0:128]
                    nc.tensor.matmul(m1[:, n * 512:(n + 1) * 512],
                                     w1sb[:], rhs,
                                     start=(t == 0), stop=True)
                r1 = rp.tile([128, 4096], dt.bfloat16, tag="r1")
                for k in range(NQ):
                    sl = slice(k * (4096 // NQ), (k + 1) * (4096 // NQ))
                    nc.scalar.activation(r1[:, sl], m1[:, sl], AF.Relu,
                                         bias=thrs[:, 0:1], scale=1.0)
                    nc.vector.scalar_tensor_tensor(m1[:, sl], r1[:, sl], 0.0,
                                                   m1[:, sl], op0=OP.is_equal,
                                                   op1=OP.mult)
                # pool on r1: [128, 32r, 128x] -> 2x2 max -> spike
                r1v = r1[:].rearrange("p (r x) -> p r x", x=128)
                q1 = qp.tile([128, 32, 64], dt.bfloat16, tag="q1")
                nc.vector.tensor_tensor(q1[:], r1v[:, :, 0:128:2],
                                        r1v[:, :, 1:128:2], op=OP.max)
                q2 = qp.tile([128, 16, 64], dt.bfloat16, tag="q2")
                nc.vector.tensor_tensor(q2[:], q1[:, 0:32:2, :], q1[:, 1:32:2, :],
                                        op=OP.max)
                s1t = sp.tile([128, 16, 64], dt.bfloat16, tag="s1t")
                nc.gpsimd.tensor_scalar(s1t[:], q2[:], 0.0, None, op0=OP.is_gt)
                sbase = (img * T + t) * C1 * PITCH1
                s1tv = s1t[:].rearrange("(g c) r x -> g c r x", g=4)
                for g in range(4):
                    dst = ap(s1d[:], sbase + (16 * g + 1) * 66 + 1,
                             [[PITCH1, 32], [66, 16], [1, 64]])
                    (nc.sync if g % 2 else nc.scalar).dma_start(dst, s1tv[g])

        # ================= P2: conv2 + spike/pool =================
        # per-image phases: m2 [64, 4096] occupies all PSUM on partitions 0-63
        GRP_SHIFT = (0, 1, 2)
        for img in range(BL):
            m2 = psum.tile([64, 4096], dt.float32, tag="mem")
            nc.vector.memset(m2[:], 0.0)
            for t in range(T):
                b2 = b2p.tile([96, 4424], dt.float32r, tag="b2")
                sbase = (img * T + t) * C1 * PITCH1
                for g in range(3):
                    d = GRP_SHIFT[g]
                    dst = b2[32 * g:32 * (g + 1), 0:4424 - d]
                    src = ap(s1d[:], sbase + d, [[PITCH1, 32], [1, 4424 - d]])
                    nc.gpsimd.dma_start(dst, src)
                for n in range(8):
                    for dy in range(3):
                        for s in range(2):
                            off = dy * 66 + (8 * n) * 66
                            v = b2[:, off:off + 8 * 66]
                            rhs = v.rearrange("p (r x) -> p r x", x=66)[:, :, 0:64]
                            lhsT = w2sb[:, (s * 3 + dy) * 64:(s * 3 + dy + 1) * 64]
                            nc.tensor.matmul(
                                m2[:, n * 512:(n + 1) * 512], lhsT, rhs,
                                start=False, stop=(dy == 2 and s == 1))
                r2 = rp.tile([64, 4096], dt.bfloat16, tag="r1")
                for k in range(NQ):
                    sl = slice(k * (4096 // NQ), (k + 1) * (4096 // NQ))
                    nc.scalar.activation(r2[:, sl], m2[:, sl], AF.Relu,
                                         bias=thrs[0:64, 1:2], scale=1.0)
                    nc.vector.scalar_tensor_tensor(m2[:, sl], r2[:, sl], 0.0,
                                                   m2[:, sl], op0=OP.is_equal,
                                                   op1=OP.mult)
                r2v = r2[:].rearrange("p (r x) -> p r x", x=64)
                q1b = qp.tile([64, 64, 32], dt.bfloat16, tag="q1")
                nc.vector.tensor_tensor(q1b[:], r2v[:, :, 0:64:2], r2v[:, :, 1:64:2],
                                        op=OP.max)
                q2b = qp.tile([64, 32, 32], dt.bfloat16, tag="q2")
                nc.vector.tensor_tensor(q2b[:], q1b[:, 0:64:2, :], q1b[:, 1:64:2, :],
                                        op=OP.max)
                s2t = sp.tile([64, 32, 32], dt.bfloat16, tag="s1t")
                nc.gpsimd.tensor_scalar(s2t[:], q2b[:], 0.0, None, op0=OP.is_gt)
                dbase = (img * T + t) * C2 * PITCH2
                dst = ap(s2d[:], dbase + 35, [[PITCH2, 64], [34, 32], [1, 32]])
                nc.sync.dma_start(dst, s2t[:])

        # ================= P3: conv3 + spike/pool =================
        m3 = psum.tile([128, 2048], dt.float32, tag="mem")
        for t in range(T):
            b3t = []
            for img in range(BL):
                b3 = b3p.tile([128, 1200], dt.float32r, tag="b3")
                sbase = (img * T + t) * C2 * PITCH2
                for g in range(2):
                    d = 34 * g
                    dst = b3[64 * g:64 * (g + 1), 0:1200 - d]
                    src = ap(s2d[:], sbase + d, [[PITCH2, 64], [1, 1200 - d]])
                    nc.gpsimd.dma_start(dst, src)
                b3t.append(b3)
            for img in range(BL):
                b3 = b3t[img]
                mslice = m3[:, img * 1024:(img + 1) * 1024]
                for n in range(2):
                    for j in range(6):
                        dx = j % 3
                        base = dx + (68 if j >= 3 else 0) + (16 * n) * 34
                        v = b3[:, base:base + 16 * 34]
                        rhs = v.rearrange("p (r x) -> p r x", x=34)[:, :, 0:32]
                        for s in range(2):
                            lhsT = w3sb[:, (s * 6 + j) * 128:(s * 6 + j + 1) * 128]
                            nc.tensor.matmul(
                                mslice[:, n * 512:(n + 1) * 512], lhsT, rhs,
                                start=(t == 0 and j == 0 and s == 0),
                                stop=(j == 5 and s == 1))
            r3 = rp.tile([128, 2048], dt.bfloat16, tag="r1")
            for k in range(2):
                sl = slice(k * 1024, (k + 1) * 1024)
                nc.scalar.activation(r3[:, sl], m3[:, sl], AF.Relu,
                                     bias=thrs[:, 2:3], scale=1.0)
                nc.vector.scalar_tensor_tensor(m3[:, sl], r3[:, sl], 0.0,
                                               m3[:, sl], op0=OP.is_equal,
                                               op1=OP.mult)
            r3v = r3[:].rearrange("p (i r x) -> p i r x", i=2, x=32)
            q1c = qp.tile([128, 2, 32, 16], dt.bfloat16, tag="q1")
            nc.vector.tensor_tensor(q1c[:], r3v[:, :, :, 0:32:2],
                                    r3v[:, :, :, 1:32:2], op=OP.max)
            q2c = qp.tile([128, 2, 16, 16], dt.bfloat16, tag="q2")
            nc.vector.tensor_tensor(q2c[:], q1c[:, :, 0:32:2, :],
                                    q1c[:, :, 1:32:2, :], op=OP.max)
            for img in range(BL):
                dst = s3_all[:, t * BL + img::T * BL]
                dstv = dst.rearrange("p (y x) -> p y x", x=16)
                nc.gpsimd.tensor_scalar(dstv, q2c[:, img], 0.0, None, op0=OP.is_gt)

        # ================= P4: FC + LIF =================
        fc = psum.tile([NCLS, T * BL], dt.float32, tag="mem")
        for px in range(256):
            nc.tensor.matmul(fc[:], qwsb[:, px * 10:(px + 1) * 10],
                             s3_all[:, px * T * BL:(px + 1) * T * BL],
                             start=(px == 0), stop=(px == 255))
        # LIF over t on [10, BL] slices (consts replicated across partitions)
        neglim = wpool.tile([128, 1], dt.float32, tag="neglim")
        nc.vector.tensor_scalar_mul(neglim[:], cons[:, 1:2], -1.0)
        fm = sp.tile([NCLS, BL], dt.float32, tag="fm")
        mem = sp.tile([NCLS, BL], dt.float32, tag="memt")
        outsb = sp.tile([NCLS, T * BL], dt.float32, tag="outsb")
        nc.vector.memset(fm[:], 0.0)
        lim_b = cons[0:NCLS, 1:2]
        nlim_b = neglim[0:NCLS, :]
        thr_b = cons[0:NCLS, 3:4]
        beta_b = cons[0:NCLS, 2:3]
        scale_b = cons[0:NCLS, 0:1]
        for t in range(T):
            cur = fc[:, t * BL:(t + 1) * BL]
            nc.vector.tensor_scalar(fm[:], fm[:], lim_b, nlim_b,
                                    op0=OP.min, op1=OP.max)
            nc.vector.scalar_tensor_tensor(mem[:], cur, scale_b, fm[:],
                                           op0=OP.mult, op1=OP.add)
            nc.vector.tensor_scalar(outsb[:, t * BL:(t + 1) * BL], mem[:],
                                    thr_b, None, op0=OP.is_gt)
            nc.vector.scalar_tensor_tensor(fm[:], mem[:], thr_b, mem[:],
                                           op0=OP.is_le, op1=OP.mult)
            nc.vector.tensor_scalar(fm[:], fm[:], beta_b, None, op0=OP.mult)
        nc.sync.dma_start(out_e[:], outsb[:])

    split_sync_waits(nc, max_waits=1)
    return nc


_NC_CACHE = {}
LAST_EXEC_NS = {}


def kernel(**inputs):
    from concourse.bass_utils import run_bass_kernel_spmd

    xs, shared = _build_host_inputs(**inputs)
    if "nc" not in _NC_CACHE:
        _NC_CACHE["nc"] = _build_bass()
    nc = _NC_CACHE["nc"]
    in_maps = [dict(shared, xs=xs[c]) for c in range(NCORES)]
    import os
    trace = bool(int(os.environ.get("FENRIR_TRACE", "0")))
    res = run_bass_kernel_spmd(nc, in_maps, list(range(NCORES)), trace=trace)
    LAST_EXEC_NS["ns"] = res.exec_time_ns
    if bool(int(os.environ.get("FENRIR_TIME", "0"))):
        import time
        best = None
        for _ in range(3):
            t0 = time.perf_counter()
            res = run_bass_kernel_spmd(nc, in_maps, list(range(NCORES)))
            dt_ = time.perf_counter() - t0
            best = dt_ if best is None or dt_ < best else best
        LAST_EXEC_NS["wall_ns"] = int(best * 1e9)
    out = np.zeros((T, B, NCLS), np.float32)
    for c in range(NCORES):
        o = res.results[c]["out"]  # [10, T*BL] col = t*BL + b
        for b in range(BL):
            out[:, c * BL + b, :] = o[:, b::BL].T
    return out


_WAITSPLIT_SRC = '''
import concourse.mybir as mybir


def split_sync_waits(nc, max_waits=1):
    n_split = 0
    for f in nc.m.functions:
        for bb in f.blocks:
            insts = bb.instructions
            out = []
            changed = False
            for inst in insts:
                si = inst.sync_info
                if si is not None and si.on_wait and len(si.on_wait) > max_waits:
                    waits = list(si.on_wait)
                    extra, keep = waits[:-max_waits], waits[-max_waits:]
                    for i in range(0, len(extra), max_waits):
                        chunk = extra[i:i + max_waits]
                        nop = mybir.InstNoOp(
                            name=nc.get_next_instruction_name(),
                            ins=[], outs=[], engine=inst.engine,
                            sync_info=mybir.SyncInfo(on_wait=chunk, on_update=[]),
                            bass_nofuse=True)
                        out.append(nop)
                        n_split += 1
                    inst.sync_info = mybir.SyncInfo(
                        on_wait=keep, on_update=list(si.on_update or []))
                    changed = True
                out.append(inst)
            if changed:
                bb.instructions = out
    return n_split
'''


if __name__ == "__main__":
    import reference as R
    inputs = R.setup_inputs()
    inputs = {k: np.asarray(v) for k, v in inputs.items()}
    out = kernel(**inputs)
    print("kernel out shape:", out.shape, "mean:", out.mean())


# revision 28
# speedup vs baseline: 1.0123x; 1.0123x over previous
"""FenrirNet (spiking CNN) Trainium2 kernel.

Data-parallel over batch: 16 images -> 8 NeuronCores x 2 images.
Per core, the T=20 scan is executed layer-phased:
  P1: conv1+spike+pool for all t (membrane m1 PSUM-resident, PE accumulates)
  P2: conv2+spike+pool for all t (m2 PSUM-resident, both images)
  P3: conv3+spike+pool for all t (m3 PSUM-resident)
  P4: quantized FC (integer-valued f32r weights, exact) + LIF readout
Convs use float32r (tf32-rate) matmuls with hi/lo mantissa-split operands, so
products are accurate to ~2^-22 — below the fp32 reordering noise floor.
Spike semantics are exact: r = relu(m - thr) (ACT), reset gate (r==0)*m (DVE),
pooled spike = (max4(r) > 0) (DVE+GPSIMD maxes on bf16 r, sign-exact).
"""
import numpy as np
from contextlib import ExitStack

# ---- model config (hardcoded; must match reference.py) ----
B, T = 16, 20
H = W = 128
C1, C2, C3 = 32, 64, 128
NCLS = 10
FC1_BITS = 4
FC1_MULT = 128.0
NCORES = 8
BL = B // NCORES  # images per core = 2

PITCH1 = 66 * 66 + 68          # padded s1 image pitch (+ tail for grp shift 66)
PITCH2 = 34 * 34 + 40          # padded s2 image pitch (+ tail for grp shift 34)
XPITCH = 130 * 130 + 12        # padded x image pitch (+ tail for dma overread)


def _mask13(a):
    return (a.view(np.uint32) & np.uint32(0xFFFFE000)).view(np.float32)


def _build_host_inputs(x, w1, w2, w3, fc_w, thr1, thr2, thr3, fc1_beta, lif_thr):
    """Precompute per-core HBM inputs (pure layout/quantization work)."""
    x = np.asarray(x, np.float32)
    w1 = np.asarray(w1, np.float32)
    w2 = np.asarray(w2, np.float32)
    w3 = np.asarray(w3, np.float32)
    fc_w = np.asarray(fc_w, np.float32)

    # exact replication of reference quantize_weight (round = half-even)
    n = float(2 ** (FC1_BITS - 1) - 1)
    scale = np.float32(np.abs(fc_w).max() / n)
    q = np.clip(np.round(fc_w / scale), -n, n).astype(np.float32)  # integer-valued

    splits = {}
    for name, wt in (("w2", w2), ("w3", w3)):
        wh = _mask13(wt)
        splits[name] = (wh, wt - wh)

    # conv1 plain-fp32 block-diag lhsT [36, 128]
    w1b = np.zeros((36, 128), np.float32)
    for g in range(4):
        for dy in range(3):
            for dx in range(3):
                w1b[g * 9 + dy * 3 + dx,
                    g * 32:(g + 1) * 32] = w1[:, 0, dy, dx]

    # conv2 lhsT, 3 dx-groups: SBUF [96, 2*3*64]: col = (s*3+dy)*64 + o
    # partition (grp=dx)*32 + c; mm(dy) covers taps (dy, dx=grp)
    w2b = np.zeros((96, 2 * 3 * 64), np.float32)
    for s, ws in enumerate(splits["w2"]):
        for dy in range(3):
            col = (s * 3 + dy) * 64
            for grp in range(3):
                w2b[grp * 32:(grp + 1) * 32, col:col + 64] = ws[:, :, dy, grp].T

    # conv3 lhsT, SBUF layout [128, 2*6*128]: col = (s*6+j)*128 + o
    w3b = np.zeros((128, 2 * 6 * 128), np.float32)
    for s, ws in enumerate(splits["w3"]):
        for j in range(6):
            dx = j % 3
            col = (s * 6 + j) * 128
            if j < 3:
                w3b[0:64, col:col + 128] = ws[:, :, 0, dx].T
                w3b[64:128, col:col + 128] = ws[:, :, 1, dx].T
            else:
                w3b[0:64, col:col + 128] = ws[:, :, 2, dx].T

    # FC: SBUF layout [128 ch, 256*10]: col = px*10 + cls; fc_in = ch*256 + px
    import ml_dtypes
    qwb = np.ascontiguousarray(
        q.reshape(NCLS, C3, 256).transpose(1, 2, 0).reshape(C3, 2560)
    ).astype(ml_dtypes.bfloat16)

    # thresholds (negated, per-partition layouts)
    nthr1 = np.tile(-np.asarray(thr1, np.float32), 4).reshape(128, 1)
    nthr2 = np.tile(-np.asarray(thr2, np.float32), 2).reshape(128, 1)
    nthr3 = (-np.asarray(thr3, np.float32)).reshape(128, 1)
    thrs = np.concatenate([nthr1, nthr2, nthr3], axis=1)  # [128, 3]

    beta = float(np.clip(np.asarray(fc1_beta, np.float32), 0.0, 1.0))
    lim = float(scale * FC1_MULT)
    consts = np.tile(np.array([[float(scale), lim, beta,
                                float(np.asarray(lif_thr))]], np.float32),
                     (128, 1))  # replicated per partition

    # padded x per core: xs[core] = [BL, T, XPITCH]
    xs = np.zeros((NCORES, BL, T, XPITCH), np.float32)
    for c in range(NCORES):
        for b in range(BL):
            for t in range(T):
                img = np.zeros((130, 130), np.float32)
                img[1:129, 1:129] = x[c * BL + b, t, 0]
                xs[c, b, t, :130 * 130] = img.reshape(-1)
    xs = xs.reshape(NCORES, BL * T * XPITCH)

    shared = {
        "w1b": w1b, "w2b": w2b, "w3b": w3b, "qwb": qwb,
        "thrs": thrs, "consts": consts,
    }
    return xs, shared


def _build_bass():
    import concourse.bass as bass
    import concourse.tile as tile
    import concourse.mybir as mybir
    import bass_rust
    _ns = {}
    exec(_WAITSPLIT_SRC, _ns)
    split_sync_waits = _ns["split_sync_waits"]

    dt = mybir.dt
    AF = mybir.ActivationFunctionType
    OP = mybir.AluOpType

    nc = bass.Bass(num_swdge_queues=4)
    xs_e = nc.declare_dram_parameter("xs", [BL * T * XPITCH], dt.float32,
                                     isOutput=False)
    w1b_e = nc.declare_dram_parameter("w1b", [36, 128], dt.float32, isOutput=False)
    w2b_e = nc.declare_dram_parameter("w2b", [96, 2 * 3 * 64], dt.float32,
                                      isOutput=False)
    w3b_e = nc.declare_dram_parameter("w3b", [128, 2 * 6 * 128], dt.float32,
                                      isOutput=False)
    qwb_e = nc.declare_dram_parameter("qwb", [128, 2560], dt.bfloat16,
                                      isOutput=False)
    thrs_e = nc.declare_dram_parameter("thrs", [128, 3], dt.float32, isOutput=False)
    con_e = nc.declare_dram_parameter("consts", [128, 4], dt.float32, isOutput=False)
    out_e = nc.declare_dram_parameter("out", [NCLS, T * BL], dt.float32, isOutput=True)

    s1d = nc.dram_tensor("s1d", [BL * T * C1 * PITCH1], dt.bfloat16)
    s2d = nc.dram_tensor("s2d", [BL * T * C2 * PITCH2 + 64], dt.bfloat16)

    def ap(tensor_ap, offset, dims):
        return bass_rust.AP(tensor=tensor_ap.tensor, offset=offset, ap=list(dims))

    with tile.TileContext(nc) as tc, ExitStack() as ctx:
        wpool = ctx.enter_context(tc.tile_pool(name="w", bufs=1))
        b1p = ctx.enter_context(tc.tile_pool(name="b1", bufs=2))
        b2p = ctx.enter_context(tc.tile_pool(name="b2", bufs=3))
        b3p = ctx.enter_context(tc.tile_pool(name="b3", bufs=3))
        rp = ctx.enter_context(tc.tile_pool(name="r", bufs=2))
        qp = ctx.enter_context(tc.tile_pool(name="q", bufs=2))
        sp = ctx.enter_context(tc.tile_pool(name="s", bufs=2))
        s3p = ctx.enter_context(tc.tile_pool(name="s3", bufs=1))
        psum = ctx.enter_context(tc.tile_pool(name="ps", bufs=1, space="PSUM"))

        # ---- static tiles (HBM already in SBUF layout; raw-bit DMAs) ----
        w1sb = wpool.tile([36, 128], dt.float32, tag="w1sb")
        nc.sync.dma_start(w1sb[:], w1b_e[:])
        w2sb = wpool.tile([96, 2 * 3 * 64], dt.float32r, tag="w2sb")
        nc.gpsimd.dma_start(w2sb[:], w2b_e[:])
        w3sb = wpool.tile([128, 2 * 6 * 128], dt.float32r, tag="w3sb")
        nc.gpsimd.dma_start(w3sb[:], w3b_e[:])
        qwsb = wpool.tile([128, 2560], dt.bfloat16, tag="qwsb")
        nc.sync.dma_start(qwsb[:], qwb_e[:])
        thrs = wpool.tile([128, 3], dt.float32, tag="thrs")
        nc.sync.dma_start(thrs[:], thrs_e[:])
        cons = wpool.tile([128, 4], dt.float32, tag="cons")
        nc.sync.dma_start(cons[:], con_e[:])
        s3_all = s3p.tile([128, 256 * T * BL], dt.bfloat16, tag="s3all")

        # zero the padded HBM scratch once (borders must be 0)
        zt = wpool.tile([128, 2048], dt.bfloat16, tag="zt")
        nc.vector.memset(zt[:], 0.0)
        for buf, tot in ((s1d, BL * T * C1 * PITCH1),
                         (s2d, BL * T * C2 * PITCH2)):
            CH = 128 * 2048
            for lo in range(0, tot, CH):
                n = min(CH, tot - lo)
                rows, rem = divmod(n, 2048)
                if rows:
                    nc.sync.dma_start(ap(buf[:], lo, [[2048, rows], [1, 2048]]),
                                      zt[0:rows, :])
                if rem:
                    nc.sync.dma_start(ap(buf[:], lo + rows * 2048, [[1, rem]]),
                                      zt[0:1, 0:rem])

        NQ = 8  # relu/reset split factor for pipelining

        # ================= P1: conv1 + spike/pool =================
        m1 = psum.tile([128, 4096], dt.float32, tag="mem")
        for img in range(BL):
            for t in range(T):
                xbase = (img * T + t) * XPITCH
                b1h = b1p.tile([36, 4164], dt.float32, tag="b1h")
                bt = b1h[:].rearrange("(g tp) c -> tp g c", tp=9)
                engs = (nc.gpsimd, nc.sync, nc.scalar)
                for dy in range(3):
                    for dx in range(3):
                        src = ap(xs_e[:], xbase + dy * 130 + dx,
                                 [[32 * 130, 4], [1, 4164]])
                        engs[(dy * 3 + dx) % 3].dma_start(bt[dy * 3 + dx], src)
                for n in range(8):
                    v = b1h[:, 520 * n:520 * n + 520]
                    rhs = v.rearrange("p (r x) -> p r x", x=130)[:, :, 0:128]
                    nc.tensor.matmul(m1[:, n * 512:(n + 1) * 512],
                                     w1sb[:], rhs,
                                     start=(t == 0), stop=True)
                r1 = rp.tile([128, 4096], dt.bfloat16, tag="r1")
                for k in range(NQ):
                    sl = slice(k * (4096 // NQ), (k + 1) * (4096 // NQ))
                    nc.scalar.activation(r1[:, sl], m1[:, sl], AF.Relu,
                                         bias=thrs[:, 0:1], scale=1.0)
                    nc.vector.scalar_tensor_tensor(m1[:, sl], r1[:, sl], 0.0,
                                                   m1[:, sl], op0=OP.is_equal,
                                                   op1=OP.mult)
                # pool on r1: [128, 32r, 128x] -> 2x2 max -> spike
                r1v = r1[:].rearrange("p (r x) -> p r x", x=128)
                q1 = qp.tile([128, 32, 64], dt.bfloat16, tag="q1")
                nc.vector.tensor_tensor(q1[:], r1v[:, :, 0:128:2],
                                        r1v[:, :, 1:128:2], op=OP.max)
                q2 = qp.tile([128, 16, 64], dt.bfloat16, tag="q2")
                nc.vector.tensor_tensor(q2[:], q1[:, 0:32:2, :], q1[:, 1:32:2, :],
                                        op=OP.max)
                s1t = sp.tile([128, 16, 64], dt.bfloat16, tag="s1t")
                nc.gpsimd.tensor_scalar(s1t[:], q2[:], 0.0, None, op0=OP.is_gt)
                sbase = (img * T + t) * C1 * PITCH1
                s1tv = s1t[:].rearrange("(g c) r x -> g c r x", g=4)
                for g in range(4):
                    dst = ap(s1d[:], sbase + (16 * g + 1) * 66 + 1,
                             [[PITCH1, 32], [66, 16], [1, 64]])
                    (nc.sync if g % 2 else nc.scalar).dma_start(dst, s1tv[g])

        # ================= P2: conv2 + spike/pool =================
        # per-image phases: m2 [64, 4096] occupies all PSUM on partitions 0-63
        GRP_SHIFT = (0, 1, 2)
        for img in range(BL):
            m2 = psum.tile([64, 4096], dt.float32, tag="mem")
            nc.vector.memset(m2[:], 0.0)
            for t in range(T):
                b2 = b2p.tile([96, 4424], dt.float32r, tag="b2")
                sbase = (img * T + t) * C1 * PITCH1
                for g in range(3):
                    d = GRP_SHIFT[g]
                    dst = b2[32 * g:32 * (g + 1), 0:4424 - d]
                    src = ap(s1d[:], sbase + d, [[PITCH1, 32], [1, 4424 - d]])
                    nc.gpsimd.dma_start(dst, src)
                for n in range(8):
                    for dy in range(3):
                        for s in range(2):
                            off = dy * 66 + (8 * n) * 66
                            v = b2[:, off:off + 8 * 66]
                            rhs = v.rearrange("p (r x) -> p r x", x=66)[:, :, 0:64]
                            lhsT = w2sb[:, (s * 3 + dy) * 64:(s * 3 + dy + 1) * 64]
                            nc.tensor.matmul(
                                m2[:, n * 512:(n + 1) * 512], lhsT, rhs,
                                start=False, stop=(dy == 2 and s == 1))
                r2 = rp.tile([64, 4096], dt.bfloat16, tag="r1")
                for k in range(NQ):
                    sl = slice(k * (4096 // NQ), (k + 1) * (4096 // NQ))
                    nc.scalar.activation(r2[:, sl], m2[:, sl], AF.Relu,
                                         bias=thrs[0:64, 1:2], scale=1.0)
                    nc.vector.scalar_tensor_tensor(m2[:, sl], r2[:, sl], 0.0,
                                                   m2[:, sl], op0=OP.is_equal,
                                                   op1=OP.mult)
                r2v = r2[:].rearrange("p (r x) -> p r x", x=64)
                q1b = qp.tile([64, 64, 32], dt.bfloat16, tag="q1")
                nc.vector.tensor_tensor(q1b[:], r2v[:, :, 0:64:2], r2v[:, :, 1:64:2],
                                        op=OP.max)
                q2b = qp.tile([64, 32, 32], dt.bfloat16, tag="q2")
                nc.vector.tensor_tensor(q2b[:], q1b[:, 0:64:2, :], q1b[:, 1:64:2, :],
                                        op=OP.max)
                s2t = sp.tile([64, 32, 32], dt.bfloat16, tag="s1t")
                nc.gpsimd.tensor_scalar(s2t[:], q2b[:], 0.0, None, op0=OP.is_gt)
                dbase = (img * T + t) * C2 * PITCH2
                dst = ap(s2d[:], dbase + 35, [[PITCH2, 64], [34, 32], [1, 32]])
                nc.sync.dma_start(dst, s2t[:])

        # ================= P3: conv3 + spike/pool =================
        m3 = psum.tile([128, 2048], dt.float32, tag="mem")
        for t in range(T):
            b3t = []
            for img in range(BL):
                b3 = b3p.tile([128, 1200], dt.float32r, tag="b3")
                sbase = (img * T + t) * C2 * PITCH2
                for g in range(2):
                    d = 34 * g
                    dst = b3[64 * g:64 * (g + 1), 0:1200 - d]
                    src = ap(s2d[:], sbase + d, [[PITCH2, 64], [1, 1200 - d]])
                    nc.gpsimd.dma_start(dst, src)
                b3t.append(b3)
            for img in range(BL):
                b3 = b3t[img]
                mslice = m3[:, img * 1024:(img + 1) * 1024]
                for n in range(2):
                    for j in range(6):
                        dx = j % 3
                        base = dx + (68 if j >= 3 else 0) + (16 * n) * 34
                        v = b3[:, base:base + 16 * 34]
                        rhs = v.rearrange("p (r x) -> p r x", x=34)[:, :, 0:32]
                        for s in range(2):
                            lhsT = w3sb[:, (s * 6 + j) * 128:(s * 6 + j + 1) * 128]
                            nc.tensor.matmul(
                                mslice[:, n * 512:(n + 1) * 512], lhsT, rhs,
                                start=(t == 0 and j == 0 and s == 0),
                                stop=(j == 5 and s == 1))
            r3 = rp.tile([128, 2048], dt.bfloat16, tag="r1")
            for k in range(4):
                sl = slice(k * 512, (k + 1) * 512)
                nc.scalar.activation(r3[:, sl], m3[:, sl], AF.Relu,
                                     bias=thrs[:, 2:3], scale=1.0)
                nc.vector.scalar_tensor_tensor(m3[:, sl], r3[:, sl], 0.0,
                                               m3[:, sl], op0=OP.is_equal,
                                               op1=OP.mult)
            r3v = r3[:].rearrange("p (i r x) -> p i r x", i=2, x=32)
            q1c = qp.tile([128, 2, 32, 16], dt.bfloat16, tag="q1")
            nc.vector.tensor_tensor(q1c[:], r3v[:, :, :, 0:32:2],
                                    r3v[:, :, :, 1:32:2], op=OP.max)
            q2c = qp.tile([128, 2, 16, 16], dt.bfloat16, tag="q2")
            nc.vector.tensor_tensor(q2c[:], q1c[:, :, 0:32:2, :],
                                    q1c[:, :, 1:32:2, :], op=OP.max)
            for img in range(BL):
                dst = s3_all[:, t * BL + img::T * BL]
                dstv = dst.rearrange("p (y x) -> p y x", x=16)
                nc.gpsimd.tensor_scalar(dstv, q2c[:, img], 0.0, None, op0=OP.is_gt)

        # ================= P4: FC + LIF =================
        fc = psum.tile([NCLS, T * BL], dt.float32, tag="mem")
        for px in range(256):
            nc.tensor.matmul(fc[:], qwsb[:, px * 10:(px + 1) * 10],
                             s3_all[:, px * T * BL:(px + 1) * T * BL],
                             start=(px == 0), stop=(px == 255))
        # LIF over t on [10, BL] slices (consts replicated across partitions)
        neglim = wpool.tile([128, 1], dt.float32, tag="neglim")
        nc.vector.tensor_scalar_mul(neglim[:], cons[:, 1:2], -1.0)
        fm = sp.tile([NCLS, BL], dt.float32, tag="fm")
        mem = sp.tile([NCLS, BL], dt.float32, tag="memt")
        outsb = sp.tile([NCLS, T * BL], dt.float32, tag="outsb")
        nc.vector.memset(fm[:], 0.0)
        lim_b = cons[0:NCLS, 1:2]
        nlim_b = neglim[0:NCLS, :]
        thr_b = cons[0:NCLS, 3:4]
        beta_b = cons[0:NCLS, 2:3]
        scale_b = cons[0:NCLS, 0:1]
        for t in range(T):
            cur = fc[:, t * BL:(t + 1) * BL]
            nc.vector.tensor_scalar(fm[:], fm[:], lim_b, nlim_b,
                                    op0=OP.min, op1=OP.max)
            nc.vector.scalar_tensor_tensor(mem[:], cur, scale_b, fm[:],
                                           op0=OP.mult, op1=OP.add)
            nc.vector.tensor_scalar(outsb[:, t * BL:(t + 1) * BL], mem[:],
                                    thr_b, None, op0=OP.is_gt)
            nc.vector.scalar_tensor_tensor(fm[:], mem[:], thr_b, mem[:],
                                           op0=OP.is_le, op1=OP.mult)
            nc.vector.tensor_scalar(fm[:], fm[:], beta_b, None, op0=OP.mult)
        nc.sync.dma_start(out_e[:], outsb[:])

    split_sync_waits(nc, max_waits=1)
    return nc


_NC_CACHE = {}
LAST_EXEC_NS = {}


def kernel(**inputs):
    from concourse.bass_utils import run_bass_kernel_spmd

    xs, shared = _build_host_inputs(**inputs)
    if "nc" not in _NC_CACHE:
        _NC_CACHE["nc"] = _build_bass()
    nc = _NC_CACHE["nc"]
    in_maps = [dict(shared, xs=xs[c]) for c in range(NCORES)]
    import os
    trace = bool(int(os.environ.get("FENRIR_TRACE", "0")))
    res = run_bass_kernel_spmd(nc, in_maps, list(range(NCORES)), trace=trace)
    LAST_EXEC_NS["ns"] = res.exec_time_ns
    if bool(int(os.environ.get("FENRIR_TIME", "0"))):
        import time
        best = None
        for _ in range(3):
            t0 = time.perf_counter()
            res = run_bass_kernel_spmd(nc, in_maps, list(range(NCORES)))
            dt_ = time.perf_counter() - t0
            best = dt_ if best is None or dt_ < best else best
        LAST_EXEC_NS["wall_ns"] = int(best * 1e9)
    out = np.zeros((T, B, NCLS), np.float32)
    for c in range(NCORES):
        o = res.results[c]["out"]  # [10, T*BL] col = t*BL + b
        for b in range(BL):
            out[:, c * BL + b, :] = o[:, b::BL].T
    return out


_WAITSPLIT_SRC = '''
import concourse.mybir as mybir


def split_sync_waits(nc, max_waits=1):
    n_split = 0
    for f in nc.m.functions:
        for bb in f.blocks:
            insts = bb.instructions
            out = []
            changed = False
            for inst in insts:
                si = inst.sync_info
                if si is not None and si.on_wait and len(si.on_wait) > max_waits:
                    waits = list(si.on_wait)
                    extra, keep = waits[:-max_waits], waits[-max_waits:]
                    for i in range(0, len(extra), max_waits):
                        chunk = extra[i:i + max_waits]
                        nop = mybir.InstNoOp(
                            name=nc.get_next_instruction_name(),
                            ins=[], outs=[], engine=inst.engine,
                            sync_info=mybir.SyncInfo(on_wait=chunk, on_update=[]),
                            bass_nofuse=True)
                        out.append(nop)
                        n_split += 1
                    inst.sync_info = mybir.SyncInfo(
                        on_wait=keep, on_update=list(si.on_update or []))
                    changed = True
                out.append(inst)
            if changed:
                bb.instructions = out
    return n_split
'''


if __name__ == "__main__":
    import reference as R
    inputs = R.setup_inputs()
    inputs = {k: np.asarray(v) for k, v in inputs.items()}
    out = kernel(**inputs)
    print("kernel out shape:", out.shape, "mean:", out.mean())
